# revision 20
# baseline (speedup 1.0000x reference)

# D2Net (nms_detection) Trainium2 Bass kernel.
#
# Strategy: batch=1, shard spatially over H across 8 NeuronCores with halo
# *recompute* (no collectives).  Each core receives a zero-padded slab of the
# image plus per-core "masked" biases; out-of-global-range rows compute to
# exactly 0 because (a) their inputs are all zero and (b) their bias column is
# masked to 0, so relu(0*w + 0) == 0.  This reproduces the reference's zero
# padding at every layer without any per-core control flow (single SPMD
# program).
#
# Convs are shift-matmuls on the PE (fp32: exact-ish), detection/localization
# are DVE/ACT/GPSIMD stencils on the 512-channel feature slab.
import os
import sys
import math
import numpy as np

for _p in (
    "/opt/trn_rl_repo",
    "/opt/pypackages",
    "/root/.axon_site/_ro/trn_rl_repo",
    "/root/.axon_site/_ro/pypackages",
):
    if os.path.isdir(_p) and _p not in sys.path:
        sys.path.insert(0, _p)

EDGE_T = 5.0
THR = (EDGE_T + 1.0) ** 2 / EDGE_T  # 7.2

_PROG_CACHE = {}


# ----------------------------------------------------------------------------
# geometry
# ----------------------------------------------------------------------------
def make_geometry(W, R):
    """W: full image H=W (multiple of 4). R: output rows per core at final res."""
    W2, W3 = W // 2, W // 4
    W4 = W3 - 1
    g = {"W": W, "R": R, "W2": W2, "W3": W3, "W4": W4}
    # rows per slab (uniform across cores) and global lo offset (fn of s)
    # lo given as (mult, add): lo = mult*s + add
    rows = {}
    rows["out"] = (R, (1, 0))
    rows["feat"] = (R + 2, (1, -1))
    rows["c42"] = (R + 6, (1, -3))
    rows["c41"] = (R + 10, (1, -5))
    rows["ap"] = (R + 14, (1, -7))
    rows["c33"] = (R + 15, (1, -7))
    rows["c32"] = (R + 17, (1, -8))
    rows["c31"] = (R + 19, (1, -9))
    rows["p2"] = (R + 21, (1, -10))
    rows["c22"] = (2 * R + 42, (2, -20))
    rows["c21"] = (2 * R + 44, (2, -21))
    rows["p1"] = (2 * R + 46, (2, -22))
    rows["c12"] = (4 * R + 92, (4, -44))
    rows["c11"] = (4 * R + 94, (4, -45))
    rows["img"] = (4 * R + 96, (4, -46))
    g["rows"] = rows
    # slab widths / core widths / left pad of core region within slab
    # (name, slab_w, core_w, padl)
    wid = {
        "img": (W + 2, W, 1),
        "c11": (W + 2, W, 1),
        "c12": (W, W, 0),
        "p1": (W2 + 2, W2, 1),
        "c21": (W2 + 2, W2, 1),
        "c22": (W2, W2, 0),
        "p2": (W3 + 2, W3, 1),
        "c31": (W3 + 2, W3, 1),
        "c32": (W3 + 2, W3, 1),
        "c33": (W3, W3, 0),
        "ap": (W4 + 4, W4, 2),
        "c41": (W4 + 4, W4, 2),
        "c42": (W4 + 4, W4, 2),
        "feat": (W4 + 2, W4, 1),
    }
    g["wid"] = wid
    # conv layer specs: (name, in, w_input_idx, Cin, Cout, dil, H_global_of_out)
    g["convs"] = [
        ("c11", "img", 1, 3, 64, 1, W),
        ("c12", "c11", 2, 64, 64, 1, W),
        ("c21", "p1", 3, 64, 128, 1, W2),
        ("c22", "c21", 4, 128, 128, 1, W2),
        ("c31", "p2", 5, 128, 256, 1, W3),
        ("c32", "c31", 6, 256, 256, 1, W3),
        ("c33", "c32", 7, 256, 256, 1, W3),
        ("c41", "ap", 8, 256, 512, 2, W4),
        ("c42", "c41", 9, 512, 512, 2, W4),
        ("feat", "c42", 10, 512, 512, 2, W4),
    ]
    g["chans"] = {
        "img": 3, "c11": 64, "c12": 64, "p1": 64, "c21": 128, "c22": 128,
        "p2": 128, "c31": 256, "c32": 256, "c33": 256, "ap": 256,
        "c41": 512, "c42": 512, "feat": 512,
    }
    return g


def core_starts(g, n_cores):
    H4 = g["W4"]
    R = g["R"]
    starts = [min(k * R, H4 - R) for k in range(n_cores)]
    return starts


# ----------------------------------------------------------------------------
# program builder
# ----------------------------------------------------------------------------
def build_program(g, max_wait_splits=None):
    import concourse.bacc as bacc
    import concourse.mybir as mybir
    import concourse.tile as tile
    import concourse.bass as bass
    from concourse.alu_op_type import AluOpType

    f32 = mybir.dt.float32
    u8 = mybir.dt.uint8
    Act = mybir.ActivationFunctionType

    nc = bacc.Bacc("TRN2", target_bir_lowering=False, debug=False)
    rows, wid, chans = g["rows"], g["wid"], g["chans"]
    W4, R = g["W4"], g["R"]

    # ---- DRAM tensors ----
    img = nc.dram_tensor("image", [3, rows["img"][0], wid["img"][0]], f32,
                         kind="ExternalInput")
    wts, bts = {}, {}
    for (name, src, wi, Cin, Cout, dil, H) in g["convs"]:
        if name == "c11":
            wts[wi] = nc.dram_tensor(f"w{wi}p", [27, 64], f32, kind="ExternalInput")
        else:
            wts[wi] = nc.dram_tensor(f"w{wi}p", [Cin, 9 * Cout], f32,
                                     kind="ExternalInput")
        bts[wi] = nc.dram_tensor(f"b{wi}m", [Cout, rows[name][0]], f32,
                                 kind="ExternalInput")
        bts[(wi, "mask")] = nc.dram_tensor(
            f"m{wi}", [Cout, rows[name][0]], f32, kind="ExternalInput")

    slabs = {"img": img}
    for nm in ("c11", "c12", "p1", "c21", "c22", "p2", "c31", "c32", "c33",
               "ap", "c41", "c42", "feat"):
        slabs[nm] = nc.dram_tensor(
            f"slab_{nm}", [chans[nm], rows[nm][0], wid[nm][0]], f32,
            kind="Internal")

    ap_mask = nc.dram_tensor("map", [256, rows["ap"][0]], f32,
                             kind="ExternalInput")
    bts["ap_mask"] = ap_mask

    feat_out = nc.dram_tensor("features", [512, R, W4], f32, kind="ExternalOutput")
    det_out = nc.dram_tensor("detected", [512, R, W4], u8, kind="ExternalOutput")
    disp_out = nc.dram_tensor("displ", [2, 512, R, W4], f32, kind="ExternalOutput")

    MAXW = wid["img"][0]

    with tile.TileContext(nc) as tc:
        # ---- zero the column pads of every slab (internal DRAM is uninit) ----
        with tc.tile_pool(name="zpool", bufs=1) as zp:
            zt = zp.tile([128, 4096], f32, tag="z")
            nc.vector.memset(zt[:], 0.0)
            for nm in slabs:
                if nm == "img":
                    continue
                slab_w, core_w, padl = wid[nm]
                C, Rn = chans[nm], rows[nm][0]
                padr = slab_w - core_w - padl
                for ci in range(max(1, C // 128)):
                    csz = min(128, C)
                    for (off, pw) in ((0, padl), (padl + core_w, padr)):
                        if pw == 0:
                            continue
                        n = Rn * pw
                        assert n <= 4096, (nm, n)
                        dst = bass.AP(
                            tensor=slabs[nm].ap().tensor,
                            offset=ci * 128 * Rn * slab_w + off,
                            ap=[[Rn * slab_w, csz], [slab_w, Rn], [1, pw]],
                        )
                        nc.sync.dma_start(out=dst, in_=zt[:csz, :n])

            # ---- conv layers + pools in sequence ----
            for (name, src, wi, Cin, Cout, dil, H) in g["convs"]:
                emit_conv(tc, nc, g, name, src, wi, Cin, Cout, dil,
                          slabs, wts, bts,
                          feat_out if name == "feat" else None)
                if name == "c12":
                    emit_pool_max(tc, nc, g, "p1", "c12", slabs)
                elif name == "c22":
                    emit_pool_max(tc, nc, g, "p2", "c22", slabs)
                elif name == "c33":
                    emit_pool_sum(tc, nc, g, "ap", "c33", slabs, ap_mask)

            # ---- detection + localization ----
            emit_detect(tc, nc, g, slabs, det_out, disp_out, zt)

    nc.compile()
    return nc


def _dt(nc):
    import concourse.mybir as mybir
    return mybir.dt.float32


def emit_conv(tc, nc, g, name, src, wi, Cin, Cout, dil, slabs, wts, bts,
              feat_out):
    import concourse.mybir as mybir
    import concourse.bass as bass
    f32 = mybir.dt.float32
    Act = mybir.ActivationFunctionType

    rows, wid = g["rows"], g["wid"]
    Rout = rows[name][0]
    Rin = rows[src][0]
    slab_w_in, core_w_in, padl_in = wid[src]
    slab_w, core_w, padl = wid[name]
    in_dram = slabs[src]
    out_dram = slabs[name]
    ncin = max(1, Cin // 128)
    ncout = max(1, Cout // 128)
    cin_sz = min(128, Cin)
    cout_sz = min(128, Cout)
    # column blocks
    nb = int(math.ceil(core_w / 448.0))
    Nb = int(math.ceil(core_w / float(nb)))
    im2row = (name == "c11")

    ring_bufs = (2 * dil + 1) + 2

    with tc.tile_pool(name=name + "_w", bufs=1) as wp, \
         tc.tile_pool(name=name + "_r", bufs=ring_bufs) as rp, \
         tc.tile_pool(name=name + "_ps", bufs=6, space="PSUM") as pp, \
         tc.tile_pool(name=name + "_o", bufs=6) as op, \
         tc.tile_pool(name=name + "_b", bufs=1) as bp:

        # weights + biases resident
        wtiles = []
        for ci in range(ncin):
            if im2row:
                t = wp.tile([27, 64], f32, tag=f"w{ci}")
                nc.sync.dma_start(out=t[:], in_=wts[wi].ap())
            else:
                t = wp.tile([cin_sz, 9 * Cout], f32, tag=f"w{ci}")
                nc.sync.dma_start(
                    out=t[:], in_=wts[wi].ap()[ci * 128: ci * 128 + cin_sz, :])
            wtiles.append(t)
        btiles = []
        mtiles = []
        for co in range(ncout):
            t = bp.tile([cout_sz, Rout], f32, tag=f"b{co}")
            nc.sync.dma_start(
                out=t[:], in_=bts[wi].ap()[co * 128: co * 128 + cout_sz, :])
            btiles.append(t)
            tm = bp.tile([cout_sz, Rout], f32, tag=f"m{co}")
            nc.sync.dma_start(
                out=tm[:],
                in_=bts[(wi, "mask")].ap()[co * 128: co * 128 + cout_sz, :])
            mtiles.append(tm)

        ring = {}

        def in_row(r, ci):
            key = (r, ci)
            if key not in ring:
                t = rp.tile([cin_sz, slab_w_in], f32, tag=f"r{ci}")
                nc.sync.dma_start(
                    out=t[:],
                    in_=in_dram.ap()[ci * 128: ci * 128 + cin_sz, r, :])
                ring[key] = t
            return ring[key]

        for r in range(Rout):
            if im2row:
                # one [27, core_w] tile: rows (di, dj, c) = img[c, r+di, x+dj]
                t27 = rp.tile([27, core_w], f32, tag="r0")
                for di in range(3):
                    src_ap = bass.AP(
                        tensor=in_dram.ap().tensor,
                        offset=(r + di) * slab_w_in,
                        ap=[[1, 3], [Rin * slab_w_in, 3], [1, core_w]],
                    )
                    nc.sync.dma_start(out=t27[di * 9:(di + 1) * 9, :],
                                      in_=src_ap)
                tap_tiles = None
            else:
                tap_tiles = [in_row(r + di * dil, ci)
                             for di in range(3) for ci in range(ncin)]
            for cb in range(nb):
                c0 = cb * Nb
                Ncols = min(Nb, core_w - c0)
                for co in range(ncout):
                    ps = pp.tile([cout_sz, Ncols], f32, tag="ps")
                    if im2row:
                        nc.tensor.matmul(ps[:], wtiles[0][:],
                                         t27[:, c0:c0 + Ncols],
                                         start=True, stop=True)
                    else:
                        k = 0
                        total = ncin * 9
                        for di in range(3):
                            for ci in range(ncin):
                                rt = tap_tiles[di * ncin + ci]
                                for dj in range(3):
                                    rhs = rt[:, c0 + dj * dil:
                                             c0 + dj * dil + Ncols]
                                    tap = di * 3 + dj
                                    lhsT = wtiles[ci][:,
                                        tap * Cout + co * 128:
                                        tap * Cout + co * 128 + cout_sz]
                                    nc.tensor.matmul(
                                        ps[:], lhsT, rhs,
                                        start=(k == 0), stop=(k == total - 1))
                                    k += 1
                    ot = op.tile([cout_sz, Ncols], f32, tag="o")
                    nc.scalar.activation(ot[:], ps[:], Act.Relu,
                                         bias=btiles[co][:, r:r + 1],
                                         scale=mtiles[co][:, r:r + 1])
                    nc.sync.dma_start(
                        out=out_dram.ap()[co * 128: co * 128 + cout_sz, r,
                                          padl + c0: padl + c0 + Ncols],
                        in_=ot[:])
                    if feat_out is not None and 1 <= r <= Rout - 2:
                        nc.sync.dma_start(
                            out=feat_out.ap()[co * 128: co * 128 + cout_sz,
                                              r - 1, :],
                            in_=ot[:])


def emit_pool_max(tc, nc, g, name, src, slabs):
    import concourse.mybir as mybir
    f32 = mybir.dt.float32
    rows, wid, chans = g["rows"], g["wid"], g["chans"]
    Rout = rows[name][0]
    slab_w_in, core_w_in, _ = wid[src]
    slab_w, core_w, padl = wid[name]
    C = chans[name]
    ncin = max(1, C // 128)
    csz = min(128, C)
    in_dram, out_dram = slabs[src], slabs[name]

    with tc.tile_pool(name=name + "_r", bufs=4) as rp, \
         tc.tile_pool(name=name + "_t", bufs=4) as tp, \
         tc.tile_pool(name=name + "_o", bufs=4) as op:
        for r in range(Rout):
            for ci in range(ncin):
                a = rp.tile([csz, slab_w_in], f32, tag=f"a{ci}")
                nc.sync.dma_start(
                    out=a[:], in_=in_dram.ap()[ci * 128: ci * 128 + csz,
                                               2 * r, :])
                b = rp.tile([csz, slab_w_in], f32, tag=f"bb{ci}")
                nc.sync.dma_start(
                    out=b[:], in_=in_dram.ap()[ci * 128: ci * 128 + csz,
                                               2 * r + 1, :])
                vm = tp.tile([csz, slab_w_in], f32, tag=f"v{ci}")
                nc.vector.tensor_max(vm[:], a[:], b[:])
                po = op.tile([csz, core_w], f32, tag=f"o{ci}")
                r3 = vm[:].rearrange("c (w two) -> c w two", two=2)
                nc.vector.tensor_max(po[:], r3[:, :, 0], r3[:, :, 1])
                nc.sync.dma_start(
                    out=out_dram.ap()[ci * 128: ci * 128 + csz, r,
                                      padl: padl + core_w],
                    in_=po[:])


def emit_pool_sum(tc, nc, g, name, src, slabs, mask_dram):
    # AvgPool2d(2, stride=1) * 4  (the 0.25 is folded into w8 on the host).
    # stride-1 window overlaps one valid row at the slab edges, so the
    # out-of-global-range rows must be forced to zero with a row mask.
    import concourse.mybir as mybir
    f32 = mybir.dt.float32
    rows, wid, chans = g["rows"], g["wid"], g["chans"]
    Rout = rows[name][0]
    slab_w_in, core_w_in, _ = wid[src]
    slab_w, core_w, padl = wid[name]
    C = chans[name]
    ncin = max(1, C // 128)
    csz = min(128, C)
    in_dram, out_dram = slabs[src], slabs[name]

    with tc.tile_pool(name=name + "_r", bufs=5) as rp, \
         tc.tile_pool(name=name + "_t", bufs=4) as tp, \
         tc.tile_pool(name=name + "_m", bufs=1) as mp, \
         tc.tile_pool(name=name + "_o", bufs=4) as op:
        mtiles = []
        for ci in range(ncin):
            tm = mp.tile([csz, Rout], f32, tag=f"mk{ci}")
            nc.sync.dma_start(
                out=tm[:], in_=mask_dram.ap()[ci * 128: ci * 128 + csz, :])
            mtiles.append(tm)
        ring = {}

        def in_row(r, ci):
            key = (r, ci)
            if key not in ring:
                t = rp.tile([csz, slab_w_in], f32, tag=f"r{ci}")
                nc.sync.dma_start(
                    out=t[:], in_=in_dram.ap()[ci * 128: ci * 128 + csz, r, :])
                ring[key] = t
            return ring[key]

        for r in range(Rout):
            for ci in range(ncin):
                a = in_row(r, ci)
                b = in_row(r + 1, ci)
                va = tp.tile([csz, slab_w_in], f32, tag=f"v{ci}")
                nc.vector.tensor_add(va[:], a[:], b[:])
                ho = tp.tile([csz, core_w], f32, tag=f"h{ci}")
                nc.vector.tensor_add(ho[:], va[:, 0:core_w], va[:, 1:core_w + 1])
                po = op.tile([csz, core_w], f32, tag=f"o{ci}")
                nc.vector.tensor_scalar_mul(po[:], ho[:], mtiles[ci][:, r:r + 1])
                nc.sync.dma_start(
                    out=out_dram.ap()[ci * 128: ci * 128 + csz, r,
                                      padl: padl + core_w],
                    in_=po[:])


def emit_detect(tc, nc, g, slabs, det_out, disp_out, zt):
    import concourse.mybir as mybir
    from concourse import bass_isa
    from concourse.alu_op_type import AluOpType as A
    f32 = mybir.dt.float32
    u8 = mybir.dt.uint8
    Act = mybir.ActivationFunctionType

    rows, wid = g["rows"], g["wid"]
    R, W4 = g["R"], g["W4"]
    Rf = rows["feat"][0]          # R + 2
    slab_w, core_w, padl = wid["feat"]   # W4+2, W4, 1
    feat = slabs["feat"]
    NCH = 4                        # 512 / 128

    with tc.tile_pool(name="dt_r", bufs=5) as rp, \
         tc.tile_pool(name="dt_t", bufs=3) as tp, \
         tc.tile_pool(name="dt_o", bufs=4) as op:
        ring = {}

        def frow(r, ci):
            key = (r, ci)
            if key not in ring:
                t = rp.tile([128, slab_w], f32, tag=f"f{ci}")
                nc.sync.dma_start(
                    out=t[:], in_=feat.ap()[ci * 128: ci * 128 + 128, r, :])
                ring[key] = t
            return ring[key]

        def tmp(tag, w=None):
            return tp.tile([128, w or core_w], f32, tag=tag, name=tag)

        for r in range(R):
            U = [frow(r, ci) for ci in range(NCH)]
            C = [frow(r + 1, ci) for ci in range(NCH)]
            D = [frow(r + 2, ci) for ci in range(NCH)]
            cc = 1 + core_w   # exclusive end of center cols
            # channelwise max across all 512 channels (center row)
            m01 = tmp("m01")
            nc.vector.tensor_max(m01[:], C[0][:, 1:cc], C[1][:, 1:cc])
            m23 = tmp("m23")
            nc.vector.tensor_max(m23[:], C[2][:, 1:cc], C[3][:, 1:cc])
            mall = tmp("mall")
            nc.vector.tensor_max(mall[:], m01[:], m23[:])
            dwmax = tmp("dwmax")
            nc.gpsimd.partition_all_reduce(dwmax[:], mall[:], 128,
                                           bass_isa.ReduceOp.max)
            for ci in range(NCH):
                u, c, d = U[ci], C[ci], D[ci]
                ctr = c[:, 1:cc]
                # second derivatives
                a1 = tmp("a1")
                nc.vector.tensor_add(a1[:], u[:, 1:cc], d[:, 1:cc])
                dii = tmp("dii")
                nc.vector.scalar_tensor_tensor(dii[:], ctr, -2.0, a1[:],
                                               A.mult, A.add)
                a2 = tmp("a2")
                nc.vector.tensor_add(a2[:], c[:, 0:cc - 1], c[:, 2:cc + 1])
                djj = tmp("djj")
                nc.vector.scalar_tensor_tensor(djj[:], ctr, -2.0, a2[:],
                                               A.mult, A.add)
                # q = (ul - ur) + (dr - dl);  dij = 0.25 q
                t2 = tmp("t2")
                nc.vector.tensor_sub(t2[:], u[:, 0:cc - 1], u[:, 2:cc + 1])
                t3 = tmp("t3")
                nc.vector.tensor_sub(t3[:], d[:, 2:cc + 1], d[:, 0:cc - 1])
                q = tmp("q")
                nc.vector.tensor_add(q[:], t2[:], t3[:])
                # det = dii*djj - 0.0625 q^2 ; tr = dii + djj
                p1 = tmp("p1")
                nc.vector.tensor_mul(p1[:], dii[:], djj[:])
                s2 = tmp("s2")
                nc.vector.tensor_mul(s2[:], q[:], q[:])
                det = tmp("det")
                nc.vector.scalar_tensor_tensor(det[:], s2[:], -0.0625, p1[:],
                                               A.mult, A.add)
                tr = tmp("tr")
                nc.vector.tensor_add(tr[:], dii[:], djj[:])
                # local max 3x3 (features >= 0 so zero-pad == -inf pad here)
                rm = tmp("rm", slab_w)
                nc.vector.tensor_max(rm[:, 0:slab_w], u[:], c[:])
                rm2 = tmp("rm2", slab_w)
                nc.vector.tensor_max(rm2[:, 0:slab_w], rm[:, 0:slab_w], d[:])
                h1 = tmp("h1")
                nc.vector.tensor_max(h1[:], rm2[:, 0:cc - 1], rm2[:, 1:cc])
                lm = tmp("lm")
                nc.vector.tensor_max(lm[:], h1[:], rm2[:, 2:cc + 1])
                is_loc = tmp("il")
                nc.vector.tensor_tensor(is_loc[:], ctr, lm[:], A.is_equal)
                is_dw = tmp("idw")
                nc.vector.tensor_tensor(is_dw[:], ctr, dwmax[:], A.is_equal)
                # not-edge: tr^2 <= THR*det  and  det > 0
                tr2 = tmp("tr2")
                nc.scalar.square(tr2[:], tr[:])
                thrd = tmp("thrd")
                nc.vector.tensor_single_scalar(thrd[:], det[:], THR, A.mult)
                cond1 = tmp("c1")
                nc.vector.tensor_tensor(cond1[:], tr2[:], thrd[:], A.is_le)
                dpos = tmp("dp")
                nc.vector.tensor_single_scalar(dpos[:], det[:], 0.0, A.is_gt)
                ne = tmp("ne")
                nc.vector.tensor_mul(ne[:], cond1[:], dpos[:])
                d1 = tmp("d1")
                nc.vector.tensor_mul(d1[:], is_loc[:], is_dw[:])
                dfin = op.tile([128, core_w], u8, tag="df", name="dfin")
                nc.vector.tensor_mul(dfin[:], d1[:], ne[:])
                nc.sync.dma_start(
                    out=det_out.ap()[ci * 128: ci * 128 + 128, r, :],
                    in_=dfin[:])
                # localization: si = A*tD + B*tJ, sj = B*tD + C*tJ with
                # A=-0.5*djj/det, B=+0.125*q/det (= +0.5*dij/det), C=-0.5*dii/det
                rdet = tmp("rd")
                with nc.allow_low_precision("reference divides in fp32"):
                    nc.vector.reciprocal(rdet[:], det[:])
                ninv00 = tmp("n00")
                nc.vector.scalar_tensor_tensor(ninv00[:], djj[:], -0.5,
                                               rdet[:], A.mult, A.mult)
                ninv01 = tmp("n01")
                nc.vector.scalar_tensor_tensor(ninv01[:], q[:], 0.125,
                                               rdet[:], A.mult, A.mult)
                ninv11 = tmp("n11")
                nc.vector.scalar_tensor_tensor(ninv11[:], dii[:], -0.5,
                                               rdet[:], A.mult, A.mult)
                tD = tmp("tD")
                nc.vector.tensor_sub(tD[:], d[:, 1:cc], u[:, 1:cc])
                tJ = tmp("tJ")
                nc.vector.tensor_sub(tJ[:], c[:, 2:cc + 1], c[:, 0:cc - 1])
                x1 = tmp("x1")
                nc.vector.tensor_mul(x1[:], ninv00[:], tD[:])
                x2 = tmp("x2")
                nc.vector.tensor_mul(x2[:], ninv01[:], tJ[:])
                si = op.tile([128, core_w], f32, tag="si", name="si")
                nc.vector.tensor_add(si[:], x1[:], x2[:])
                nc.sync.dma_start(
                    out=disp_out.ap()[0, ci * 128: ci * 128 + 128, r, :],
                    in_=si[:])
                y1 = tmp("y1")
                nc.vector.tensor_mul(y1[:], ninv01[:], tD[:])
                y2 = tmp("y2")
                nc.vector.tensor_mul(y2[:], ninv11[:], tJ[:])
                sj = op.tile([128, core_w], f32, tag="sj", name="sj")
                nc.vector.tensor_add(sj[:], y1[:], y2[:])
                nc.sync.dma_start(
                    out=disp_out.ap()[1, ci * 128: ci * 128 + 128, r, :],
                    in_=sj[:])


# ----------------------------------------------------------------------------
# host side
# ----------------------------------------------------------------------------
def _pack_weights(ws):
    """ws: dict i -> (Cout, Cin, 3, 3) float32. Returns packed dict."""
    out = {}
    for i, w in ws.items():
        w = np.asarray(w, np.float32)
        if i == 8:
            w = w * 0.25  # fold AvgPool2d(2,1)'s 1/4 into conv4_1
        Cout, Cin = w.shape[0], w.shape[1]
        if i == 1:
            # im2row packing: row p = di*9 + dj*3 + c
            p = np.transpose(w, (2, 3, 1, 0)).reshape(27, Cout)
            out[i] = np.ascontiguousarray(p, np.float32)
        else:
            # [Cin, 9*Cout]: col = tap*Cout + cout, tap = di*3+dj
            p = np.transpose(w, (2, 3, 0, 1)).reshape(9 * Cout, Cin)
            out[i] = np.ascontiguousarray(p.T, np.float32)
    return out


def _masked_biases(g, bs, s):
    """per-core masked biases: dict i -> [Cout, Rout]"""
    rows = g["rows"]
    H_of = {}
    for (name, src, wi, Cin, Cout, dil, H) in g["convs"]:
        H_of[wi] = (name, H)
    out = {}
    masks = {}
    for i, b in bs.items():
        name, H = H_of[i]
        Rn, (mult, add) = rows[name]
        lo = mult * s + add
        mask = np.array([1.0 if 0 <= lo + r < H else 0.0 for r in range(Rn)],
                        np.float32)
        out[i] = np.ascontiguousarray(
            np.asarray(b, np.float32)[:, None] * mask[None, :], np.float32)
        Cout = np.asarray(b).shape[0]
        masks[i] = np.ascontiguousarray(
            np.broadcast_to(mask[None, :], (Cout, Rn)), np.float32)
    return out, masks


def _ap_mask(g, s):
    Rn, (mult, add) = g["rows"]["ap"]
    lo = mult * s + add
    H = g["W4"]
    mask = np.array([1.0 if 0 <= lo + r < H else 0.0 for r in range(Rn)],
                    np.float32)
    return np.ascontiguousarray(
        np.broadcast_to(mask[None, :], (256, Rn)), np.float32)


def _image_slab(g, image, s):
    """image: (3, W, W). Returns zero-padded [3, Rimg, W+2] slab."""
    W = g["W"]
    Rimg, (mult, add) = g["rows"]["img"]
    lo = mult * s + add
    slab = np.zeros((3, Rimg, W + 2), np.float32)
    a = max(0, lo)
    b = min(W, lo + Rimg)
    if b > a:
        slab[:, a - lo: b - lo, 1: 1 + W] = image[:, a:b, :]
    return slab


def run_d2net(image, ws, bs, W=768, R=24, n_cores=8):
    from concourse.bass_utils import run_bass_kernel_spmd

    g = make_geometry(W, R)
    key = (W, R)
    if key not in _PROG_CACHE:
        _PROG_CACHE[key] = build_program(g)
    nc = _PROG_CACHE[key]

    starts = core_starts(g, n_cores)
    wp = _pack_weights(ws)
    in_maps = []
    for k in range(n_cores):
        s = starts[k]
        m = {"image": _image_slab(g, image, s)}
        for i in range(1, 11):
            m[f"w{i}p"] = wp[i]
        bm, mk = _masked_biases(g, bs, s)
        for i in range(1, 11):
            m[f"b{i}m"] = bm[i]
            m[f"m{i}"] = mk[i]
        m["map"] = _ap_mask(g, s)
        in_maps.append(m)

    res = run_bass_kernel_spmd(nc, in_maps, core_ids=list(range(n_cores)))

    W4 = g["W4"]
    features = np.zeros((1, 512, W4, W4), np.float32)
    detected = np.zeros((1, 512, W4, W4), np.bool_)
    displ = np.zeros((1, 2, 512, W4, W4), np.float32)
    for k in range(n_cores):
        s = starts[k]
        f = res.results[k]["features"]          # [512, R, W4]
        d = res.results[k]["detected"]          # [512, R, W4] u8
        p = res.results[k]["displ"]             # [2, 512, R, W4]
        prev_end = 0 if k == 0 else starts[k - 1] + R
        off = prev_end - s  # rows of this slab already covered by previous core
        features[0, :, s + off: s + R, :] = f[:, off:, :]
        detected[0, :, s + off: s + R, :] = d[:, off:, :].astype(np.bool_)
        displ[0, :, :, s + off: s + R, :] = p[:, :, off:, :]
    return features, detected, displ


def kernel(**inputs):
    image = np.asarray(inputs["image"], np.float32)[0]  # (3, 768, 768)
    ws = {i: np.asarray(inputs[f"w{i}"], np.float32) for i in range(1, 11)}
    bs = {i: np.asarray(inputs[f"b{i}"], np.float32) for i in range(1, 11)}
    return run_d2net(image, ws, bs, W=768, R=24, n_cores=8)


# revision 21
# speedup vs baseline: 56.7968x; 56.7968x over previous

# D2Net (nms_detection) Trainium2 Bass kernel.
#
# Strategy: batch=1, shard spatially over H across 8 NeuronCores with halo
# *recompute* (no collectives).  Each core receives a zero-padded slab of the
# image plus per-core "masked" biases; out-of-global-range rows compute to
# exactly 0 because (a) their inputs are all zero and (b) their bias column is
# masked to 0, so relu(0*w + 0) == 0.  This reproduces the reference's zero
# padding at every layer without any per-core control flow (single SPMD
# program).
#
# Convs are shift-matmuls on the PE (fp32: exact-ish), detection/localization
# are DVE/ACT/GPSIMD stencils on the 512-channel feature slab.
import os
import sys
import math
import numpy as np

for _p in (
    "/opt/trn_rl_repo",
    "/opt/pypackages",
    "/root/.axon_site/_ro/trn_rl_repo",
    "/root/.axon_site/_ro/pypackages",
):
    if os.path.isdir(_p) and _p not in sys.path:
        sys.path.insert(0, _p)

EDGE_T = 5.0
THR = (EDGE_T + 1.0) ** 2 / EDGE_T  # 7.2

_PROG_CACHE = {}


# ----------------------------------------------------------------------------
# geometry
# ----------------------------------------------------------------------------
def make_geometry(W, R):
    """W: full image H=W (multiple of 4). R: output rows per core at final res."""
    W2, W3 = W // 2, W // 4
    W4 = W3 - 1
    g = {"W": W, "R": R, "W2": W2, "W3": W3, "W4": W4}
    # rows per slab (uniform across cores) and global lo offset (fn of s)
    # lo given as (mult, add): lo = mult*s + add
    rows = {}
    rows["out"] = (R, (1, 0))
    rows["feat"] = (R + 2, (1, -1))
    rows["c42"] = (R + 6, (1, -3))
    rows["c41"] = (R + 10, (1, -5))
    rows["ap"] = (R + 14, (1, -7))
    rows["c33"] = (R + 15, (1, -7))
    rows["c32"] = (R + 17, (1, -8))
    rows["c31"] = (R + 19, (1, -9))
    rows["p2"] = (R + 21, (1, -10))
    rows["c22"] = (2 * R + 42, (2, -20))
    rows["c21"] = (2 * R + 44, (2, -21))
    rows["p1"] = (2 * R + 46, (2, -22))
    rows["c12"] = (4 * R + 92, (4, -44))
    rows["c11"] = (4 * R + 94, (4, -45))
    rows["img"] = (4 * R + 96, (4, -46))
    g["rows"] = rows
    # slab widths / core widths / left pad of core region within slab
    # (name, slab_w, core_w, padl)
    wid = {
        "img": (W + 2, W, 1),
        "c11": (W + 2, W, 1),
        "c12": (W, W, 0),
        "p1": (W2 + 2, W2, 1),
        "c21": (W2 + 2, W2, 1),
        "c22": (W2, W2, 0),
        "p2": (W3 + 2, W3, 1),
        "c31": (W3 + 2, W3, 1),
        "c32": (W3 + 2, W3, 1),
        "c33": (W3, W3, 0),
        "ap": (W4 + 4, W4, 2),
        "c41": (W4 + 4, W4, 2),
        "c42": (W4 + 4, W4, 2),
        "feat": (W4 + 2, W4, 1),
    }
    g["wid"] = wid
    # conv layer specs: (name, in, w_input_idx, Cin, Cout, dil, H_global_of_out)
    g["convs"] = [
        ("c11", "img", 1, 3, 64, 1, W),
        ("c12", "c11", 2, 64, 64, 1, W),
        ("c21", "p1", 3, 64, 128, 1, W2),
        ("c22", "c21", 4, 128, 128, 1, W2),
        ("c31", "p2", 5, 128, 256, 1, W3),
        ("c32", "c31", 6, 256, 256, 1, W3),
        ("c33", "c32", 7, 256, 256, 1, W3),
        ("c41", "ap", 8, 256, 512, 2, W4),
        ("c42", "c41", 9, 512, 512, 2, W4),
        ("feat", "c42", 10, 512, 512, 2, W4),
    ]
    g["chans"] = {
        "img": 3, "c11": 64, "c12": 64, "p1": 64, "c21": 128, "c22": 128,
        "p2": 128, "c31": 256, "c32": 256, "c33": 256, "ap": 256,
        "c41": 512, "c42": 512, "feat": 512,
    }
    return g


def core_starts(g, n_cores):
    H4 = g["W4"]
    R = g["R"]
    starts = [min(k * R, H4 - R) for k in range(n_cores)]
    return starts


# ----------------------------------------------------------------------------
# program builder
# ----------------------------------------------------------------------------
def build_program(g, max_wait_splits=None):
    import concourse.bacc as bacc
    import concourse.mybir as mybir
    import concourse.tile as tile
    import concourse.bass as bass
    from concourse.alu_op_type import AluOpType

    f32 = mybir.dt.float32
    u8 = mybir.dt.uint8
    Act = mybir.ActivationFunctionType

    nc = bacc.Bacc("TRN2", target_bir_lowering=False, debug=False)
    rows, wid, chans = g["rows"], g["wid"], g["chans"]
    W4, R = g["W4"], g["R"]

    # ---- DRAM tensors ----
    img = nc.dram_tensor("image", [3, rows["img"][0], wid["img"][0]], f32,
                         kind="ExternalInput")
    wts, bts = {}, {}
    for (name, src, wi, Cin, Cout, dil, H) in g["convs"]:
        if name == "c11":
            wts[wi] = nc.dram_tensor(f"w{wi}p", [27, 64], f32, kind="ExternalInput")
        else:
            wts[wi] = nc.dram_tensor(f"w{wi}p", [Cin, 9 * Cout], f32,
                                     kind="ExternalInput")
        bts[wi] = nc.dram_tensor(f"b{wi}m", [Cout, rows[name][0]], f32,
                                 kind="ExternalInput")
        bts[(wi, "mask")] = nc.dram_tensor(
            f"m{wi}", [Cout, rows[name][0]], f32, kind="ExternalInput")

    slabs = {"img": img}
    for nm in ("c11", "c12", "p1", "c21", "c22", "p2", "c31", "c32", "c33",
               "ap", "c41", "c42", "feat"):
        slabs[nm] = nc.dram_tensor(
            f"slab_{nm}", [chans[nm], rows[nm][0], wid[nm][0]], f32,
            kind="Internal")

    ap_mask = nc.dram_tensor("map", [256, rows["ap"][0]], f32,
                             kind="ExternalInput")
    bts["ap_mask"] = ap_mask

    feat_out = nc.dram_tensor("features", [512, R, W4], f32, kind="ExternalOutput")
    det_out = nc.dram_tensor("detected", [512, R, W4], u8, kind="ExternalOutput")
    disp_out = nc.dram_tensor("displ", [2, 512, R, W4], f32, kind="ExternalOutput")

    MAXW = wid["img"][0]

    with tile.TileContext(nc) as tc:
        # ---- zero the column pads of every slab (internal DRAM is uninit) ----
        with tc.tile_pool(name="zpool", bufs=1) as zp:
            zt = zp.tile([128, 4096], f32, tag="z")
            nc.vector.memset(zt[:], 0.0)
            for nm in slabs:
                if nm == "img":
                    continue
                slab_w, core_w, padl = wid[nm]
                C, Rn = chans[nm], rows[nm][0]
                padr = slab_w - core_w - padl
                for ci in range(max(1, C // 128)):
                    csz = min(128, C)
                    for (off, pw) in ((0, padl), (padl + core_w, padr)):
                        if pw == 0:
                            continue
                        n = Rn * pw
                        assert n <= 4096, (nm, n)
                        dst = bass.AP(
                            tensor=slabs[nm].ap().tensor,
                            offset=ci * 128 * Rn * slab_w + off,
                            ap=[[Rn * slab_w, csz], [slab_w, Rn], [1, pw]],
                        )
                        nc.sync.dma_start(out=dst, in_=zt[:csz, :n])

            # ---- conv layers + pools in sequence ----
            for (name, src, wi, Cin, Cout, dil, H) in g["convs"]:
                emit_conv(tc, nc, g, name, src, wi, Cin, Cout, dil,
                          slabs, wts, bts,
                          feat_out if name == "feat" else None)
                if name == "c12":
                    emit_pool_max(tc, nc, g, "p1", "c12", slabs)
                elif name == "c22":
                    emit_pool_max(tc, nc, g, "p2", "c22", slabs)
                elif name == "c33":
                    emit_pool_sum(tc, nc, g, "ap", "c33", slabs, ap_mask)

            # ---- detection + localization ----
            emit_detect(tc, nc, g, slabs, det_out, disp_out, zt)

    nc.compile()
    return nc


def _dt(nc):
    import concourse.mybir as mybir
    return mybir.dt.float32


def emit_conv(tc, nc, g, name, src, wi, Cin, Cout, dil, slabs, wts, bts,
              feat_out):
    import concourse.mybir as mybir
    import concourse.bass as bass
    f32 = mybir.dt.float32
    Act = mybir.ActivationFunctionType

    rows, wid = g["rows"], g["wid"]
    Rout = rows[name][0]
    Rin = rows[src][0]
    slab_w_in, core_w_in, padl_in = wid[src]
    slab_w, core_w, padl = wid[name]
    in_dram = slabs[src]
    out_dram = slabs[name]
    ncin = max(1, Cin // 128)
    ncout = max(1, Cout // 128)
    cin_sz = min(128, Cin)
    cout_sz = min(128, Cout)
    # column blocks
    nb = int(math.ceil(core_w / 448.0))
    Nb = int(math.ceil(core_w / float(nb)))
    im2row = (name == "c11")

    ring_bufs = (2 * dil + 1) + 2

    with tc.tile_pool(name=name + "_w", bufs=1) as wp, \
         tc.tile_pool(name=name + "_r", bufs=ring_bufs) as rp, \
         tc.tile_pool(name=name + "_ps", bufs=6, space="PSUM") as pp, \
         tc.tile_pool(name=name + "_o", bufs=6) as op, \
         tc.tile_pool(name=name + "_b", bufs=1) as bp:

        # weights + biases resident
        wtiles = []
        for ci in range(ncin):
            if im2row:
                t = wp.tile([27, 64], f32, tag=f"w{ci}")
                nc.sync.dma_start(out=t[:], in_=wts[wi].ap())
            else:
                t = wp.tile([cin_sz, 9 * Cout], f32, tag=f"w{ci}")
                nc.sync.dma_start(
                    out=t[:], in_=wts[wi].ap()[ci * 128: ci * 128 + cin_sz, :])
            wtiles.append(t)
        btiles = []
        mtiles = []
        for co in range(ncout):
            t = bp.tile([cout_sz, Rout], f32, tag=f"b{co}")
            nc.sync.dma_start(
                out=t[:], in_=bts[wi].ap()[co * 128: co * 128 + cout_sz, :])
            btiles.append(t)
            tm = bp.tile([cout_sz, Rout], f32, tag=f"m{co}")
            nc.sync.dma_start(
                out=tm[:],
                in_=bts[(wi, "mask")].ap()[co * 128: co * 128 + cout_sz, :])
            mtiles.append(tm)

        ring = {}

        def in_row(r, ci):
            key = (r, ci)
            if key not in ring:
                t = rp.tile([cin_sz, slab_w_in], f32, tag=f"r{ci}")
                nc.sync.dma_start(
                    out=t[:],
                    in_=in_dram.ap()[ci * 128: ci * 128 + cin_sz, r, :])
                ring[key] = t
            return ring[key]

        for r in range(Rout):
            if im2row:
                # one [27, core_w] tile: rows (di, dj, c) = img[c, r+di, x+dj]
                t27 = rp.tile([27, core_w], f32, tag="r0")
                for di in range(3):
                    src_ap = bass.AP(
                        tensor=in_dram.ap().tensor,
                        offset=(r + di) * slab_w_in,
                        ap=[[1, 3], [Rin * slab_w_in, 3], [1, core_w]],
                    )
                    nc.sync.dma_start(out=t27[di * 9:(di + 1) * 9, :],
                                      in_=src_ap)
                tap_tiles = None
            else:
                tap_tiles = [in_row(r + di * dil, ci)
                             for di in range(3) for ci in range(ncin)]
            for cb in range(nb):
                c0 = cb * Nb
                Ncols = min(Nb, core_w - c0)
                for co in range(ncout):
                    ps = pp.tile([cout_sz, Ncols], f32, tag="ps")
                    if im2row:
                        nc.tensor.matmul(ps[:], wtiles[0][:],
                                         t27[:, c0:c0 + Ncols],
                                         start=True, stop=True)
                    else:
                        k = 0
                        total = ncin * 9
                        for di in range(3):
                            for ci in range(ncin):
                                rt = tap_tiles[di * ncin + ci]
                                for dj in range(3):
                                    rhs = rt[:, c0 + dj * dil:
                                             c0 + dj * dil + Ncols]
                                    tap = di * 3 + dj
                                    lhsT = wtiles[ci][:,
                                        tap * Cout + co * 128:
                                        tap * Cout + co * 128 + cout_sz]
                                    nc.tensor.matmul(
                                        ps[:], lhsT, rhs,
                                        start=(k == 0), stop=(k == total - 1))
                                    k += 1
                    ot = op.tile([cout_sz, Ncols], f32, tag="o")
                    nc.scalar.activation(ot[:], ps[:], Act.Relu,
                                         bias=btiles[co][:, r:r + 1],
                                         scale=mtiles[co][:, r:r + 1])
                    nc.sync.dma_start(
                        out=out_dram.ap()[co * 128: co * 128 + cout_sz, r,
                                          padl + c0: padl + c0 + Ncols],
                        in_=ot[:])
                    if feat_out is not None and 1 <= r <= Rout - 2:
                        nc.sync.dma_start(
                            out=feat_out.ap()[co * 128: co * 128 + cout_sz,
                                              r - 1, :],
                            in_=ot[:])


def emit_pool_max(tc, nc, g, name, src, slabs):
    import concourse.mybir as mybir
    f32 = mybir.dt.float32
    rows, wid, chans = g["rows"], g["wid"], g["chans"]
    Rout = rows[name][0]
    slab_w_in, core_w_in, _ = wid[src]
    slab_w, core_w, padl = wid[name]
    C = chans[name]
    ncin = max(1, C // 128)
    csz = min(128, C)
    in_dram, out_dram = slabs[src], slabs[name]

    with tc.tile_pool(name=name + "_r", bufs=4) as rp, \
         tc.tile_pool(name=name + "_t", bufs=4) as tp, \
         tc.tile_pool(name=name + "_o", bufs=4) as op:
        for r in range(Rout):
            for ci in range(ncin):
                a = rp.tile([csz, slab_w_in], f32, tag=f"a{ci}")
                nc.sync.dma_start(
                    out=a[:], in_=in_dram.ap()[ci * 128: ci * 128 + csz,
                                               2 * r, :])
                b = rp.tile([csz, slab_w_in], f32, tag=f"bb{ci}")
                nc.sync.dma_start(
                    out=b[:], in_=in_dram.ap()[ci * 128: ci * 128 + csz,
                                               2 * r + 1, :])
                vm = tp.tile([csz, slab_w_in], f32, tag=f"v{ci}")
                nc.vector.tensor_max(vm[:], a[:], b[:])
                po = op.tile([csz, core_w], f32, tag=f"o{ci}")
                r3 = vm[:].rearrange("c (w two) -> c w two", two=2)
                nc.vector.tensor_max(po[:], r3[:, :, 0], r3[:, :, 1])
                nc.sync.dma_start(
                    out=out_dram.ap()[ci * 128: ci * 128 + csz, r,
                                      padl: padl + core_w],
                    in_=po[:])


def emit_pool_sum(tc, nc, g, name, src, slabs, mask_dram):
    # AvgPool2d(2, stride=1) * 4  (the 0.25 is folded into w8 on the host).
    # stride-1 window overlaps one valid row at the slab edges, so the
    # out-of-global-range rows must be forced to zero with a row mask.
    import concourse.mybir as mybir
    f32 = mybir.dt.float32
    rows, wid, chans = g["rows"], g["wid"], g["chans"]
    Rout = rows[name][0]
    slab_w_in, core_w_in, _ = wid[src]
    slab_w, core_w, padl = wid[name]
    C = chans[name]
    ncin = max(1, C // 128)
    csz = min(128, C)
    in_dram, out_dram = slabs[src], slabs[name]

    with tc.tile_pool(name=name + "_r", bufs=5) as rp, \
         tc.tile_pool(name=name + "_t", bufs=4) as tp, \
         tc.tile_pool(name=name + "_m", bufs=1) as mp, \
         tc.tile_pool(name=name + "_o", bufs=4) as op:
        mtiles = []
        for ci in range(ncin):
            tm = mp.tile([csz, Rout], f32, tag=f"mk{ci}")
            nc.sync.dma_start(
                out=tm[:], in_=mask_dram.ap()[ci * 128: ci * 128 + csz, :])
            mtiles.append(tm)
        ring = {}

        def in_row(r, ci):
            key = (r, ci)
            if key not in ring:
                t = rp.tile([csz, slab_w_in], f32, tag=f"r{ci}")
                nc.sync.dma_start(
                    out=t[:], in_=in_dram.ap()[ci * 128: ci * 128 + csz, r, :])
                ring[key] = t
            return ring[key]

        for r in range(Rout):
            for ci in range(ncin):
                a = in_row(r, ci)
                b = in_row(r + 1, ci)
                va = tp.tile([csz, slab_w_in], f32, tag=f"v{ci}")
                nc.vector.tensor_add(va[:], a[:], b[:])
                ho = tp.tile([csz, core_w], f32, tag=f"h{ci}")
                nc.vector.tensor_add(ho[:], va[:, 0:core_w], va[:, 1:core_w + 1])
                po = op.tile([csz, core_w], f32, tag=f"o{ci}")
                nc.vector.tensor_scalar_mul(po[:], ho[:], mtiles[ci][:, r:r + 1])
                nc.sync.dma_start(
                    out=out_dram.ap()[ci * 128: ci * 128 + csz, r,
                                      padl: padl + core_w],
                    in_=po[:])


def emit_detect(tc, nc, g, slabs, det_out, disp_out, zt):
    import concourse.mybir as mybir
    from concourse import bass_isa
    from concourse.alu_op_type import AluOpType as A
    f32 = mybir.dt.float32
    u8 = mybir.dt.uint8
    Act = mybir.ActivationFunctionType

    rows, wid = g["rows"], g["wid"]
    R, W4 = g["R"], g["W4"]
    Rf = rows["feat"][0]          # R + 2
    slab_w, core_w, padl = wid["feat"]   # W4+2, W4, 1
    feat = slabs["feat"]
    NCH = 4                        # 512 / 128

    with tc.tile_pool(name="dt_r", bufs=5) as rp, \
         tc.tile_pool(name="dt_t", bufs=3) as tp, \
         tc.tile_pool(name="dt_o", bufs=4) as op:
        ring = {}

        def frow(r, ci):
            key = (r, ci)
            if key not in ring:
                t = rp.tile([128, slab_w], f32, tag=f"f{ci}")
                nc.sync.dma_start(
                    out=t[:], in_=feat.ap()[ci * 128: ci * 128 + 128, r, :])
                ring[key] = t
            return ring[key]

        def tmp(tag, w=None):
            return tp.tile([128, w or core_w], f32, tag=tag, name=tag)

        for r in range(R):
            U = [frow(r, ci) for ci in range(NCH)]
            C = [frow(r + 1, ci) for ci in range(NCH)]
            D = [frow(r + 2, ci) for ci in range(NCH)]
            cc = 1 + core_w   # exclusive end of center cols
            # channelwise max across all 512 channels (center row)
            m01 = tmp("m01")
            nc.vector.tensor_max(m01[:], C[0][:, 1:cc], C[1][:, 1:cc])
            m23 = tmp("m23")
            nc.vector.tensor_max(m23[:], C[2][:, 1:cc], C[3][:, 1:cc])
            mall = tmp("mall")
            nc.vector.tensor_max(mall[:], m01[:], m23[:])
            dwmax = tmp("dwmax")
            nc.gpsimd.partition_all_reduce(dwmax[:], mall[:], 128,
                                           bass_isa.ReduceOp.max)
            for ci in range(NCH):
                u, c, d = U[ci], C[ci], D[ci]
                ctr = c[:, 1:cc]
                # second derivatives
                a1 = tmp("a1")
                nc.vector.tensor_add(a1[:], u[:, 1:cc], d[:, 1:cc])
                dii = tmp("dii")
                nc.vector.scalar_tensor_tensor(dii[:], ctr, -2.0, a1[:],
                                               A.mult, A.add)
                a2 = tmp("a2")
                nc.vector.tensor_add(a2[:], c[:, 0:cc - 1], c[:, 2:cc + 1])
                djj = tmp("djj")
                nc.vector.scalar_tensor_tensor(djj[:], ctr, -2.0, a2[:],
                                               A.mult, A.add)
                # q = (ul - ur) + (dr - dl);  dij = 0.25 q
                t2 = tmp("t2")
                nc.vector.tensor_sub(t2[:], u[:, 0:cc - 1], u[:, 2:cc + 1])
                t3 = tmp("t3")
                nc.vector.tensor_sub(t3[:], d[:, 2:cc + 1], d[:, 0:cc - 1])
                q = tmp("q")
                nc.vector.tensor_add(q[:], t2[:], t3[:])
                # det = dii*djj - 0.0625 q^2 ; tr = dii + djj
                p1 = tmp("p1")
                nc.vector.tensor_mul(p1[:], dii[:], djj[:])
                s2 = tmp("s2")
                nc.vector.tensor_mul(s2[:], q[:], q[:])
                det = tmp("det")
                nc.vector.scalar_tensor_tensor(det[:], s2[:], -0.0625, p1[:],
                                               A.mult, A.add)
                tr = tmp("tr")
                nc.vector.tensor_add(tr[:], dii[:], djj[:])
                # local max 3x3 (features >= 0 so zero-pad == -inf pad here)
                rm = tmp("rm", slab_w)
                nc.vector.tensor_max(rm[:, 0:slab_w], u[:], c[:])
                rm2 = tmp("rm2", slab_w)
                nc.vector.tensor_max(rm2[:, 0:slab_w], rm[:, 0:slab_w], d[:])
                h1 = tmp("h1")
                nc.vector.tensor_max(h1[:], rm2[:, 0:cc - 1], rm2[:, 1:cc])
                lm = tmp("lm")
                nc.vector.tensor_max(lm[:], h1[:], rm2[:, 2:cc + 1])
                is_loc = tmp("il")
                nc.vector.tensor_tensor(is_loc[:], ctr, lm[:], A.is_equal)
                is_dw = tmp("idw")
                nc.vector.tensor_tensor(is_dw[:], ctr, dwmax[:], A.is_equal)
                # not-edge: tr^2 <= THR*det  and  det > 0
                tr2 = tmp("tr2")
                nc.scalar.square(tr2[:], tr[:])
                thrd = tmp("thrd")
                nc.vector.tensor_single_scalar(thrd[:], det[:], THR, A.mult)
                cond1 = tmp("c1")
                nc.vector.tensor_tensor(cond1[:], tr2[:], thrd[:], A.is_le)
                dpos = tmp("dp")
                nc.vector.tensor_single_scalar(dpos[:], det[:], 0.0, A.is_gt)
                ne = tmp("ne")
                nc.vector.tensor_mul(ne[:], cond1[:], dpos[:])
                d1 = tmp("d1")
                nc.vector.tensor_mul(d1[:], is_loc[:], is_dw[:])
                dfin = op.tile([128, core_w], u8, tag="df", name="dfin")
                nc.vector.tensor_mul(dfin[:], d1[:], ne[:])
                nc.sync.dma_start(
                    out=det_out.ap()[ci * 128: ci * 128 + 128, r, :],
                    in_=dfin[:])
                # localization: si = A*tD + B*tJ, sj = B*tD + C*tJ with
                # A=-0.5*djj/det, B=+0.125*q/det (= +0.5*dij/det), C=-0.5*dii/det
                # det is pre-scaled by 2^64 (exact) so reciprocal() cannot
                # overflow to inf at denormal dets; the 2^64 is folded back
                # into the (power-of-two, exact) coefficients below.
                SC = float(2.0 ** 64)
                dets = tmp("dsc")
                nc.vector.tensor_single_scalar(dets[:], det[:], SC, A.mult)
                rdet = tmp("rd")
                with nc.allow_low_precision("reference divides in fp32"):
                    nc.vector.reciprocal(rdet[:], dets[:])
                ninv00 = tmp("n00")
                nc.vector.scalar_tensor_tensor(ninv00[:], djj[:], -0.5 * SC,
                                               rdet[:], A.mult, A.mult)
                ninv01 = tmp("n01")
                nc.vector.scalar_tensor_tensor(ninv01[:], q[:], 0.125 * SC,
                                               rdet[:], A.mult, A.mult)
                ninv11 = tmp("n11")
                nc.vector.scalar_tensor_tensor(ninv11[:], dii[:], -0.5 * SC,
                                               rdet[:], A.mult, A.mult)
                tD = tmp("tD")
                nc.vector.tensor_sub(tD[:], d[:, 1:cc], u[:, 1:cc])
                tJ = tmp("tJ")
                nc.vector.tensor_sub(tJ[:], c[:, 2:cc + 1], c[:, 0:cc - 1])
                x1 = tmp("x1")
                nc.vector.tensor_mul(x1[:], ninv00[:], tD[:])
                x2 = tmp("x2")
                nc.vector.tensor_mul(x2[:], ninv01[:], tJ[:])
                si = op.tile([128, core_w], f32, tag="si", name="si")
                nc.vector.tensor_add(si[:], x1[:], x2[:])
                nc.sync.dma_start(
                    out=disp_out.ap()[0, ci * 128: ci * 128 + 128, r, :],
                    in_=si[:])
                y1 = tmp("y1")
                nc.vector.tensor_mul(y1[:], ninv01[:], tD[:])
                y2 = tmp("y2")
                nc.vector.tensor_mul(y2[:], ninv11[:], tJ[:])
                sj = op.tile([128, core_w], f32, tag="sj", name="sj")
                nc.vector.tensor_add(sj[:], y1[:], y2[:])
                nc.sync.dma_start(
                    out=disp_out.ap()[1, ci * 128: ci * 128 + 128, r, :],
                    in_=sj[:])


# ----------------------------------------------------------------------------
# host side
# ----------------------------------------------------------------------------
def _pack_weights(ws):
    """ws: dict i -> (Cout, Cin, 3, 3) float32. Returns packed dict."""
    out = {}
    for i, w in ws.items():
        w = np.asarray(w, np.float32)
        if i == 8:
            w = w * 0.25  # fold AvgPool2d(2,1)'s 1/4 into conv4_1
        Cout, Cin = w.shape[0], w.shape[1]
        if i == 1:
            # im2row packing: row p = di*9 + dj*3 + c
            p = np.transpose(w, (2, 3, 1, 0)).reshape(27, Cout)
            out[i] = np.ascontiguousarray(p, np.float32)
        else:
            # [Cin, 9*Cout]: col = tap*Cout + cout, tap = di*3+dj
            p = np.transpose(w, (2, 3, 0, 1)).reshape(9 * Cout, Cin)
            out[i] = np.ascontiguousarray(p.T, np.float32)
    return out


def _masked_biases(g, bs, s):
    """per-core masked biases: dict i -> [Cout, Rout]"""
    rows = g["rows"]
    H_of = {}
    for (name, src, wi, Cin, Cout, dil, H) in g["convs"]:
        H_of[wi] = (name, H)
    out = {}
    masks = {}
    for i, b in bs.items():
        name, H = H_of[i]
        Rn, (mult, add) = rows[name]
        lo = mult * s + add
        mask = np.array([1.0 if 0 <= lo + r < H else 0.0 for r in range(Rn)],
                        np.float32)
        out[i] = np.ascontiguousarray(
            np.asarray(b, np.float32)[:, None] * mask[None, :], np.float32)
        Cout = np.asarray(b).shape[0]
        masks[i] = np.ascontiguousarray(
            np.broadcast_to(mask[None, :], (Cout, Rn)), np.float32)
    return out, masks


def _ap_mask(g, s):
    Rn, (mult, add) = g["rows"]["ap"]
    lo = mult * s + add
    H = g["W4"]
    mask = np.array([1.0 if 0 <= lo + r < H else 0.0 for r in range(Rn)],
                    np.float32)
    return np.ascontiguousarray(
        np.broadcast_to(mask[None, :], (256, Rn)), np.float32)


def _image_slab(g, image, s):
    """image: (3, W, W). Returns zero-padded [3, Rimg, W+2] slab."""
    W = g["W"]
    Rimg, (mult, add) = g["rows"]["img"]
    lo = mult * s + add
    slab = np.zeros((3, Rimg, W + 2), np.float32)
    a = max(0, lo)
    b = min(W, lo + Rimg)
    if b > a:
        slab[:, a - lo: b - lo, 1: 1 + W] = image[:, a:b, :]
    return slab


def run_d2net(image, ws, bs, W=768, R=24, n_cores=8):
    from concourse.bass_utils import run_bass_kernel_spmd

    g = make_geometry(W, R)
    key = (W, R)
    if key not in _PROG_CACHE:
        _PROG_CACHE[key] = build_program(g)
    nc = _PROG_CACHE[key]

    starts = core_starts(g, n_cores)
    wp = _pack_weights(ws)
    in_maps = []
    for k in range(n_cores):
        s = starts[k]
        m = {"image": _image_slab(g, image, s)}
        for i in range(1, 11):
            m[f"w{i}p"] = wp[i]
        bm, mk = _masked_biases(g, bs, s)
        for i in range(1, 11):
            m[f"b{i}m"] = bm[i]
            m[f"m{i}"] = mk[i]
        m["map"] = _ap_mask(g, s)
        in_maps.append(m)

    res = run_bass_kernel_spmd(nc, in_maps, core_ids=list(range(n_cores)))

    W4 = g["W4"]
    features = np.zeros((1, 512, W4, W4), np.float32)
    detected = np.zeros((1, 512, W4, W4), np.bool_)
    displ = np.zeros((1, 2, 512, W4, W4), np.float32)
    for k in range(n_cores):
        s = starts[k]
        f = res.results[k]["features"]          # [512, R, W4]
        d = res.results[k]["detected"]          # [512, R, W4] u8
        p = res.results[k]["displ"]             # [2, 512, R, W4]
        prev_end = 0 if k == 0 else starts[k - 1] + R
        off = prev_end - s  # rows of this slab already covered by previous core
        features[0, :, s + off: s + R, :] = f[:, off:, :]
        detected[0, :, s + off: s + R, :] = d[:, off:, :].astype(np.bool_)
        displ[0, :, :, s + off: s + R, :] = p[:, :, off:, :]
    return features, detected, displ


def kernel(**inputs):
    image = np.asarray(inputs["image"], np.float32)[0]  # (3, 768, 768)
    ws = {i: np.asarray(inputs[f"w{i}"], np.float32) for i in range(1, 11)}
    bs = {i: np.asarray(inputs[f"b{i}"], np.float32) for i in range(1, 11)}
    return run_d2net(image, ws, bs, W=768, R=24, n_cores=8)


# revision 23
# speedup vs baseline: 66.9090x; 1.1780x over previous

# D2Net (nms_detection) Trainium2 Bass kernel.
#
# Strategy: batch=1, shard spatially over H across 8 NeuronCores with halo
# *recompute* (no collectives).  Each core receives a zero-padded slab of the
# image plus per-core "masked" biases; out-of-global-range rows compute to
# exactly 0 because (a) their inputs are all zero and (b) their bias column is
# masked to 0, so relu(0*w + 0) == 0.  This reproduces the reference's zero
# padding at every layer without any per-core control flow (single SPMD
# program).
#
# Convs are shift-matmuls on the PE (fp32: exact-ish), detection/localization
# are DVE/ACT/GPSIMD stencils on the 512-channel feature slab.
import os
import sys
import math
import numpy as np

for _p in (
    "/opt/trn_rl_repo",
    "/opt/pypackages",
    "/root/.axon_site/_ro/trn_rl_repo",
    "/root/.axon_site/_ro/pypackages",
):
    if os.path.isdir(_p) and _p not in sys.path:
        sys.path.insert(0, _p)

EDGE_T = 5.0
THR = (EDGE_T + 1.0) ** 2 / EDGE_T  # 7.2

_PROG_CACHE = {}


# ----------------------------------------------------------------------------
# geometry
# ----------------------------------------------------------------------------
def make_geometry(W, R):
    """W: full image H=W (multiple of 4). R: output rows per core at final res."""
    W2, W3 = W // 2, W // 4
    W4 = W3 - 1
    g = {"W": W, "R": R, "W2": W2, "W3": W3, "W4": W4}
    # rows per slab (uniform across cores) and global lo offset (fn of s)
    # lo given as (mult, add): lo = mult*s + add
    rows = {}
    rows["out"] = (R, (1, 0))
    rows["feat"] = (R + 2, (1, -1))
    rows["c42"] = (R + 6, (1, -3))
    rows["c41"] = (R + 10, (1, -5))
    rows["ap"] = (R + 14, (1, -7))
    rows["c33"] = (R + 15, (1, -7))
    rows["c32"] = (R + 17, (1, -8))
    rows["c31"] = (R + 19, (1, -9))
    rows["p2"] = (R + 21, (1, -10))
    rows["c22"] = (2 * R + 42, (2, -20))
    rows["c21"] = (2 * R + 44, (2, -21))
    rows["p1"] = (2 * R + 46, (2, -22))
    rows["c12"] = (4 * R + 92, (4, -44))
    rows["c11"] = (4 * R + 94, (4, -45))
    rows["img"] = (4 * R + 96, (4, -46))
    g["rows"] = rows
    # slab widths / core widths / left pad of core region within slab
    # (name, slab_w, core_w, padl)
    wid = {
        "img": (W + 2, W, 1),
        "c11": (W + 2, W, 1),
        "c12": (W, W, 0),
        "p1": (W2 + 2, W2, 1),
        "c21": (W2 + 2, W2, 1),
        "c22": (W2, W2, 0),
        "p2": (W3 + 2, W3, 1),
        "c31": (W3 + 2, W3, 1),
        "c32": (W3 + 2, W3, 1),
        "c33": (W3, W3, 0),
        "ap": (W4 + 4, W4, 2),
        "c41": (W4 + 4, W4, 2),
        "c42": (W4 + 4, W4, 2),
        "feat": (W4 + 2, W4, 1),
    }
    g["wid"] = wid
    # conv layer specs: (name, in, w_input_idx, Cin, Cout, dil, H_global_of_out)
    g["convs"] = [
        ("c11", "img", 1, 3, 64, 1, W),
        ("c12", "c11", 2, 64, 64, 1, W),
        ("c21", "p1", 3, 64, 128, 1, W2),
        ("c22", "c21", 4, 128, 128, 1, W2),
        ("c31", "p2", 5, 128, 256, 1, W3),
        ("c32", "c31", 6, 256, 256, 1, W3),
        ("c33", "c32", 7, 256, 256, 1, W3),
        ("c41", "ap", 8, 256, 512, 2, W4),
        ("c42", "c41", 9, 512, 512, 2, W4),
        ("feat", "c42", 10, 512, 512, 2, W4),
    ]
    g["chans"] = {
        "img": 3, "c11": 64, "c12": 64, "p1": 64, "c21": 128, "c22": 128,
        "p2": 128, "c31": 256, "c32": 256, "c33": 256, "ap": 256,
        "c41": 512, "c42": 512, "feat": 512,
    }
    return g


def core_starts(g, n_cores):
    H4 = g["W4"]
    R = g["R"]
    starts = [min(k * R, H4 - R) for k in range(n_cores)]
    return starts


# ----------------------------------------------------------------------------
# program builder
# ----------------------------------------------------------------------------
def build_program(g, max_wait_splits=None):
    import concourse.bacc as bacc
    import concourse.mybir as mybir
    import concourse.tile as tile
    import concourse.bass as bass
    from concourse.alu_op_type import AluOpType

    f32 = mybir.dt.float32
    u8 = mybir.dt.uint8
    Act = mybir.ActivationFunctionType

    nc = bacc.Bacc("TRN2", target_bir_lowering=False, debug=False)
    rows, wid, chans = g["rows"], g["wid"], g["chans"]
    W4, R = g["W4"], g["R"]

    # ---- DRAM tensors ----
    img = nc.dram_tensor("image", [3, rows["img"][0], wid["img"][0]], f32,
                         kind="ExternalInput")
    wts, bts = {}, {}
    for (name, src, wi, Cin, Cout, dil, H) in g["convs"]:
        if name == "c11":
            wts[wi] = nc.dram_tensor(f"w{wi}p", [27, 64], f32, kind="ExternalInput")
        else:
            wts[wi] = nc.dram_tensor(f"w{wi}p", [Cin, 9 * Cout], f32,
                                     kind="ExternalInput")
        bts[wi] = nc.dram_tensor(f"b{wi}m", [Cout, rows[name][0]], f32,
                                 kind="ExternalInput")
        bts[(wi, "mask")] = nc.dram_tensor(
            f"m{wi}", [Cout, rows[name][0]], f32, kind="ExternalInput")

    slabs = {"img": img}
    for nm in ("c11", "c12", "p1", "c21", "c22", "p2", "c31", "c32", "c33",
               "ap", "c41", "c42", "feat"):
        slabs[nm] = nc.dram_tensor(
            f"slab_{nm}", [chans[nm], rows[nm][0], wid[nm][0]], f32,
            kind="Internal")

    ap_mask = nc.dram_tensor("map", [256, rows["ap"][0]], f32,
                             kind="ExternalInput")
    bts["ap_mask"] = ap_mask

    feat_out = nc.dram_tensor("features", [512, R, W4], f32, kind="ExternalOutput")
    det_out = nc.dram_tensor("detected", [512, R, W4], u8, kind="ExternalOutput")
    disp_out = nc.dram_tensor("displ", [2, 512, R, W4], f32, kind="ExternalOutput")

    MAXW = wid["img"][0]

    with tile.TileContext(nc) as tc:
        # ---- zero the column pads of every slab (internal DRAM is uninit) ----
        with tc.tile_pool(name="zpool", bufs=1) as zp:
            zt = zp.tile([128, 4096], f32, tag="z")
            nc.vector.memset(zt[:], 0.0)
            for nm in slabs:
                if nm == "img":
                    continue
                slab_w, core_w, padl = wid[nm]
                C, Rn = chans[nm], rows[nm][0]
                padr = slab_w - core_w - padl
                for ci in range(max(1, C // 128)):
                    csz = min(128, C)
                    for (off, pw) in ((0, padl), (padl + core_w, padr)):
                        if pw == 0:
                            continue
                        n = Rn * pw
                        assert n <= 4096, (nm, n)
                        dst = bass.AP(
                            tensor=slabs[nm].ap().tensor,
                            offset=ci * 128 * Rn * slab_w + off,
                            ap=[[Rn * slab_w, csz], [slab_w, Rn], [1, pw]],
                        )
                        nc.sync.dma_start(out=dst, in_=zt[:csz, :n])

            # ---- conv layers + pools in sequence ----
            for (name, src, wi, Cin, Cout, dil, H) in g["convs"]:
                emit_conv(tc, nc, g, name, src, wi, Cin, Cout, dil,
                          slabs, wts, bts,
                          feat_out if name == "feat" else None)
                if name == "c12":
                    emit_pool_max(tc, nc, g, "p1", "c12", slabs)
                elif name == "c22":
                    emit_pool_max(tc, nc, g, "p2", "c22", slabs)
                elif name == "c33":
                    emit_pool_sum(tc, nc, g, "ap", "c33", slabs, ap_mask)

            # ---- detection + localization ----
            emit_detect(tc, nc, g, slabs, det_out, disp_out, zt)

    nc.compile()
    return nc


def _dt(nc):
    import concourse.mybir as mybir
    return mybir.dt.float32


def emit_conv(tc, nc, g, name, src, wi, Cin, Cout, dil, slabs, wts, bts,
              feat_out):
    import concourse.mybir as mybir
    import concourse.bass as bass
    f32 = mybir.dt.float32
    Act = mybir.ActivationFunctionType

    rows, wid = g["rows"], g["wid"]
    Rout = rows[name][0]
    Rin = rows[src][0]
    slab_w_in, core_w_in, padl_in = wid[src]
    slab_w, core_w, padl = wid[name]
    in_dram = slabs[src]
    out_dram = slabs[name]
    ncin = max(1, Cin // 128)
    ncout = max(1, Cout // 128)
    cin_sz = min(128, Cin)
    cout_sz = min(128, Cout)
    # column blocks
    nb = int(math.ceil(core_w / 448.0))
    Nb = int(math.ceil(core_w / float(nb)))
    im2row = (name == "c11")

    ring_bufs = (2 * dil + 1) + 2

    with tc.tile_pool(name=name + "_w", bufs=1) as wp, \
         tc.tile_pool(name=name + "_r", bufs=ring_bufs) as rp, \
         tc.tile_pool(name=name + "_ps", bufs=6, space="PSUM") as pp, \
         tc.tile_pool(name=name + "_o", bufs=6) as op, \
         tc.tile_pool(name=name + "_b", bufs=1) as bp:

        # weights + biases resident
        wtiles = []
        for ci in range(ncin):
            if im2row:
                t = wp.tile([27, 64], f32, tag=f"w{ci}")
                nc.sync.dma_start(out=t[:], in_=wts[wi].ap())
            else:
                t = wp.tile([cin_sz, 9 * Cout], f32, tag=f"w{ci}")
                nc.sync.dma_start(
                    out=t[:], in_=wts[wi].ap()[ci * 128: ci * 128 + cin_sz, :])
            wtiles.append(t)
        btiles = []
        mtiles = []
        for co in range(ncout):
            t = bp.tile([cout_sz, Rout], f32, tag=f"b{co}")
            nc.sync.dma_start(
                out=t[:], in_=bts[wi].ap()[co * 128: co * 128 + cout_sz, :])
            btiles.append(t)
            tm = bp.tile([cout_sz, Rout], f32, tag=f"m{co}")
            nc.sync.dma_start(
                out=tm[:],
                in_=bts[(wi, "mask")].ap()[co * 128: co * 128 + cout_sz, :])
            mtiles.append(tm)

        ring = {}

        def in_row(r, ci):
            key = (r, ci)
            if key not in ring:
                t = rp.tile([cin_sz, slab_w_in], f32, tag=f"r{ci}")
                nc.sync.dma_start(
                    out=t[:],
                    in_=in_dram.ap()[ci * 128: ci * 128 + cin_sz, r, :])
                ring[key] = t
            return ring[key]

        for r in range(Rout):
            if im2row:
                # one [27, core_w] tile: rows (di, dj, c) = img[c, r+di, x+dj]
                t27 = rp.tile([27, core_w], f32, tag="r0")
                for di in range(3):
                    src_ap = bass.AP(
                        tensor=in_dram.ap().tensor,
                        offset=(r + di) * slab_w_in,
                        ap=[[1, 3], [Rin * slab_w_in, 3], [1, core_w]],
                    )
                    nc.sync.dma_start(out=t27[di * 9:(di + 1) * 9, :],
                                      in_=src_ap)
                tap_tiles = None
            else:
                tap_tiles = [in_row(r + di * dil, ci)
                             for di in range(3) for ci in range(ncin)]
            for cb in range(nb):
                c0 = cb * Nb
                Ncols = min(Nb, core_w - c0)
                for co in range(ncout):
                    ps = pp.tile([cout_sz, Ncols], f32, tag="ps")
                    if im2row:
                        nc.tensor.matmul(ps[:], wtiles[0][:],
                                         t27[:, c0:c0 + Ncols],
                                         start=True, stop=True)
                    else:
                        k = 0
                        total = ncin * 9
                        for di in range(3):
                            for ci in range(ncin):
                                rt = tap_tiles[di * ncin + ci]
                                for dj in range(3):
                                    rhs = rt[:, c0 + dj * dil:
                                             c0 + dj * dil + Ncols]
                                    tap = di * 3 + dj
                                    lhsT = wtiles[ci][:,
                                        tap * Cout + co * 128:
                                        tap * Cout + co * 128 + cout_sz]
                                    nc.tensor.matmul(
                                        ps[:], lhsT, rhs,
                                        start=(k == 0), stop=(k == total - 1))
                                    k += 1
                    ot = op.tile([cout_sz, Ncols], f32, tag="o")
                    nc.scalar.activation(ot[:], ps[:], Act.Relu,
                                         bias=btiles[co][:, r:r + 1],
                                         scale=mtiles[co][:, r:r + 1])
                    nc.sync.dma_start(
                        out=out_dram.ap()[co * 128: co * 128 + cout_sz, r,
                                          padl + c0: padl + c0 + Ncols],
                        in_=ot[:])
                    if feat_out is not None and 1 <= r <= Rout - 2:
                        nc.sync.dma_start(
                            out=feat_out.ap()[co * 128: co * 128 + cout_sz,
                                              r - 1, :],
                            in_=ot[:])


def emit_pool_max(tc, nc, g, name, src, slabs):
    import concourse.mybir as mybir
    f32 = mybir.dt.float32
    rows, wid, chans = g["rows"], g["wid"], g["chans"]
    Rout = rows[name][0]
    slab_w_in, core_w_in, _ = wid[src]
    slab_w, core_w, padl = wid[name]
    C = chans[name]
    ncin = max(1, C // 128)
    csz = min(128, C)
    in_dram, out_dram = slabs[src], slabs[name]

    with tc.tile_pool(name=name + "_r", bufs=4) as rp, \
         tc.tile_pool(name=name + "_t", bufs=4) as tp, \
         tc.tile_pool(name=name + "_o", bufs=4) as op:
        for r in range(Rout):
            for ci in range(ncin):
                a = rp.tile([csz, slab_w_in], f32, tag=f"a{ci}")
                nc.sync.dma_start(
                    out=a[:], in_=in_dram.ap()[ci * 128: ci * 128 + csz,
                                               2 * r, :])
                b = rp.tile([csz, slab_w_in], f32, tag=f"bb{ci}")
                nc.sync.dma_start(
                    out=b[:], in_=in_dram.ap()[ci * 128: ci * 128 + csz,
                                               2 * r + 1, :])
                vm = tp.tile([csz, slab_w_in], f32, tag=f"v{ci}")
                nc.vector.tensor_max(vm[:], a[:], b[:])
                po = op.tile([csz, core_w], f32, tag=f"o{ci}")
                r3 = vm[:].rearrange("c (w two) -> c w two", two=2)
                nc.vector.tensor_max(po[:], r3[:, :, 0], r3[:, :, 1])
                nc.sync.dma_start(
                    out=out_dram.ap()[ci * 128: ci * 128 + csz, r,
                                      padl: padl + core_w],
                    in_=po[:])


def emit_pool_sum(tc, nc, g, name, src, slabs, mask_dram):
    # AvgPool2d(2, stride=1) * 4  (the 0.25 is folded into w8 on the host).
    # stride-1 window overlaps one valid row at the slab edges, so the
    # out-of-global-range rows must be forced to zero with a row mask.
    import concourse.mybir as mybir
    f32 = mybir.dt.float32
    rows, wid, chans = g["rows"], g["wid"], g["chans"]
    Rout = rows[name][0]
    slab_w_in, core_w_in, _ = wid[src]
    slab_w, core_w, padl = wid[name]
    C = chans[name]
    ncin = max(1, C // 128)
    csz = min(128, C)
    in_dram, out_dram = slabs[src], slabs[name]

    with tc.tile_pool(name=name + "_r", bufs=5) as rp, \
         tc.tile_pool(name=name + "_t", bufs=4) as tp, \
         tc.tile_pool(name=name + "_m", bufs=1) as mp, \
         tc.tile_pool(name=name + "_o", bufs=4) as op:
        mtiles = []
        for ci in range(ncin):
            tm = mp.tile([csz, Rout], f32, tag=f"mk{ci}")
            nc.sync.dma_start(
                out=tm[:], in_=mask_dram.ap()[ci * 128: ci * 128 + csz, :])
            mtiles.append(tm)
        ring = {}

        def in_row(r, ci):
            key = (r, ci)
            if key not in ring:
                t = rp.tile([csz, slab_w_in], f32, tag=f"r{ci}")
                nc.sync.dma_start(
                    out=t[:], in_=in_dram.ap()[ci * 128: ci * 128 + csz, r, :])
                ring[key] = t
            return ring[key]

        for r in range(Rout):
            for ci in range(ncin):
                a = in_row(r, ci)
                b = in_row(r + 1, ci)
                va = tp.tile([csz, slab_w_in], f32, tag=f"v{ci}")
                nc.vector.tensor_add(va[:], a[:], b[:])
                ho = tp.tile([csz, core_w], f32, tag=f"h{ci}")
                nc.vector.tensor_add(ho[:], va[:, 0:core_w], va[:, 1:core_w + 1])
                po = op.tile([csz, core_w], f32, tag=f"o{ci}")
                nc.vector.tensor_scalar_mul(po[:], ho[:], mtiles[ci][:, r:r + 1])
                nc.sync.dma_start(
                    out=out_dram.ap()[ci * 128: ci * 128 + csz, r,
                                      padl: padl + core_w],
                    in_=po[:])


def emit_detect(tc, nc, g, slabs, det_out, disp_out, zt):
    import concourse.mybir as mybir
    from concourse import bass_isa
    from concourse.alu_op_type import AluOpType as A
    f32 = mybir.dt.float32
    u8 = mybir.dt.uint8
    Act = mybir.ActivationFunctionType

    rows, wid = g["rows"], g["wid"]
    R, W4 = g["R"], g["W4"]
    Rf = rows["feat"][0]          # R + 2
    slab_w, core_w, padl = wid["feat"]   # W4+2, W4, 1
    feat = slabs["feat"]
    NCH = 4                        # 512 / 128

    with tc.tile_pool(name="dt_r", bufs=5) as rp, \
         tc.tile_pool(name="dt_t", bufs=3) as tp, \
         tc.tile_pool(name="dt_o", bufs=4) as op:
        ring = {}

        def frow(r, ci):
            key = (r, ci)
            if key not in ring:
                t = rp.tile([128, slab_w], f32, tag=f"f{ci}")
                nc.sync.dma_start(
                    out=t[:], in_=feat.ap()[ci * 128: ci * 128 + 128, r, :])
                ring[key] = t
            return ring[key]

        def tmp(tag, w=None):
            return tp.tile([128, w or core_w], f32, tag=tag, name=tag)

        for r in range(R):
            U = [frow(r, ci) for ci in range(NCH)]
            C = [frow(r + 1, ci) for ci in range(NCH)]
            D = [frow(r + 2, ci) for ci in range(NCH)]
            cc = 1 + core_w   # exclusive end of center cols
            # channelwise max across all 512 channels (center row)
            m01 = tmp("m01")
            nc.vector.tensor_max(m01[:], C[0][:, 1:cc], C[1][:, 1:cc])
            m23 = tmp("m23")
            nc.vector.tensor_max(m23[:], C[2][:, 1:cc], C[3][:, 1:cc])
            mall = tmp("mall")
            nc.vector.tensor_max(mall[:], m01[:], m23[:])
            dwmax = tmp("dwmax")
            nc.gpsimd.partition_all_reduce(dwmax[:], mall[:], 128,
                                           bass_isa.ReduceOp.max)
            for ci in range(NCH):
                u, c, d = U[ci], C[ci], D[ci]
                ctr = c[:, 1:cc]
                # second derivatives
                a1 = tmp("a1")
                nc.vector.tensor_add(a1[:], u[:, 1:cc], d[:, 1:cc])
                dii = tmp("dii")
                nc.vector.scalar_tensor_tensor(dii[:], ctr, -2.0, a1[:],
                                               A.mult, A.add)
                a2 = tmp("a2")
                nc.vector.tensor_add(a2[:], c[:, 0:cc - 1], c[:, 2:cc + 1])
                djj = tmp("djj")
                nc.vector.scalar_tensor_tensor(djj[:], ctr, -2.0, a2[:],
                                               A.mult, A.add)
                # q = (ul - ur) + (dr - dl);  dij = 0.25 q
                t2 = tmp("t2")
                nc.vector.tensor_sub(t2[:], u[:, 0:cc - 1], u[:, 2:cc + 1])
                t3 = tmp("t3")
                nc.vector.tensor_sub(t3[:], d[:, 2:cc + 1], d[:, 0:cc - 1])
                q = tmp("q")
                nc.vector.tensor_add(q[:], t2[:], t3[:])
                # det = dii*djj - dij*dij with dij = 0.25*q rounded first,
                # exactly matching the reference's rounding order
                dij = tmp("dij")
                nc.vector.tensor_single_scalar(dij[:], q[:], 0.25, A.mult)
                p1 = tmp("p1")
                nc.vector.tensor_mul(p1[:], dii[:], djj[:])
                s2 = tmp("s2")
                nc.vector.tensor_mul(s2[:], dij[:], dij[:])
                det = tmp("det")
                nc.vector.tensor_sub(det[:], p1[:], s2[:])
                tr = tmp("tr")
                nc.vector.tensor_add(tr[:], dii[:], djj[:])
                # local max 3x3 (features >= 0 so zero-pad == -inf pad here)
                rm = tmp("rm", slab_w)
                nc.vector.tensor_max(rm[:, 0:slab_w], u[:], c[:])
                rm2 = tmp("rm2", slab_w)
                nc.vector.tensor_max(rm2[:, 0:slab_w], rm[:, 0:slab_w], d[:])
                h1 = tmp("h1")
                nc.vector.tensor_max(h1[:], rm2[:, 0:cc - 1], rm2[:, 1:cc])
                lm = tmp("lm")
                nc.vector.tensor_max(lm[:], h1[:], rm2[:, 2:cc + 1])
                is_loc = tmp("il")
                nc.vector.tensor_tensor(is_loc[:], ctr, lm[:], A.is_equal)
                is_dw = tmp("idw")
                nc.vector.tensor_tensor(is_dw[:], ctr, dwmax[:], A.is_equal)
                # not-edge: tr^2 <= THR*det  and  det > 0
                tr2 = tmp("tr2")
                nc.scalar.square(tr2[:], tr[:])
                thrd = tmp("thrd")
                nc.vector.tensor_single_scalar(thrd[:], det[:], THR, A.mult)
                cond1 = tmp("c1")
                nc.vector.tensor_tensor(cond1[:], tr2[:], thrd[:], A.is_le)
                dpos = tmp("dp")
                nc.vector.tensor_single_scalar(dpos[:], det[:], 0.0, A.is_gt)
                ne = tmp("ne")
                nc.vector.tensor_mul(ne[:], cond1[:], dpos[:])
                d1 = tmp("d1")
                nc.vector.tensor_mul(d1[:], is_loc[:], is_dw[:])
                dfin = op.tile([128, core_w], u8, tag="df", name="dfin")
                nc.vector.tensor_mul(dfin[:], d1[:], ne[:])
                nc.sync.dma_start(
                    out=det_out.ap()[ci * 128: ci * 128 + 128, r, :],
                    in_=dfin[:])
                # localization: si = A*tD + B*tJ, sj = B*tD + C*tJ with
                # A=-0.5*djj/det, B=+0.125*q/det (= +0.5*dij/det), C=-0.5*dii/det
                # det is pre-scaled by 2^64 (exact) so reciprocal() cannot
                # overflow to inf at denormal dets; the 2^64 is folded back
                # into the (power-of-two, exact) coefficients below.
                SC = float(2.0 ** 64)
                dets = tmp("dsc")
                nc.vector.tensor_single_scalar(dets[:], det[:], SC, A.mult)
                rdet = tmp("rd")
                with nc.allow_low_precision("reference divides in fp32"):
                    nc.vector.reciprocal(rdet[:], dets[:])
                ninv00 = tmp("n00")
                nc.vector.scalar_tensor_tensor(ninv00[:], djj[:], -0.5 * SC,
                                               rdet[:], A.mult, A.mult)
                ninv01 = tmp("n01")
                nc.vector.scalar_tensor_tensor(ninv01[:], dij[:], 0.5 * SC,
                                               rdet[:], A.mult, A.mult)
                ninv11 = tmp("n11")
                nc.vector.scalar_tensor_tensor(ninv11[:], dii[:], -0.5 * SC,
                                               rdet[:], A.mult, A.mult)
                tD = tmp("tD")
                nc.vector.tensor_sub(tD[:], d[:, 1:cc], u[:, 1:cc])
                tJ = tmp("tJ")
                nc.vector.tensor_sub(tJ[:], c[:, 2:cc + 1], c[:, 0:cc - 1])
                x1 = tmp("x1")
                nc.vector.tensor_mul(x1[:], ninv00[:], tD[:])
                x2 = tmp("x2")
                nc.vector.tensor_mul(x2[:], ninv01[:], tJ[:])
                si = op.tile([128, core_w], f32, tag="si", name="si")
                nc.vector.tensor_add(si[:], x1[:], x2[:])
                nc.sync.dma_start(
                    out=disp_out.ap()[0, ci * 128: ci * 128 + 128, r, :],
                    in_=si[:])
                y1 = tmp("y1")
                nc.vector.tensor_mul(y1[:], ninv01[:], tD[:])
                y2 = tmp("y2")
                nc.vector.tensor_mul(y2[:], ninv11[:], tJ[:])
                sj = op.tile([128, core_w], f32, tag="sj", name="sj")
                nc.vector.tensor_add(sj[:], y1[:], y2[:])
                nc.sync.dma_start(
                    out=disp_out.ap()[1, ci * 128: ci * 128 + 128, r, :],
                    in_=sj[:])


# ----------------------------------------------------------------------------
# host side
# ----------------------------------------------------------------------------
def _pack_weights(ws):
    """ws: dict i -> (Cout, Cin, 3, 3) float32. Returns packed dict."""
    out = {}
    for i, w in ws.items():
        w = np.asarray(w, np.float32)
        if i == 8:
            w = w * 0.25  # fold AvgPool2d(2,1)'s 1/4 into conv4_1
        Cout, Cin = w.shape[0], w.shape[1]
        if i == 1:
            # im2row packing: row p = di*9 + dj*3 + c
            p = np.transpose(w, (2, 3, 1, 0)).reshape(27, Cout)
            out[i] = np.ascontiguousarray(p, np.float32)
        else:
            # [Cin, 9*Cout]: col = tap*Cout + cout, tap = di*3+dj
            p = np.transpose(w, (2, 3, 0, 1)).reshape(9 * Cout, Cin)
            out[i] = np.ascontiguousarray(p.T, np.float32)
    return out


def _masked_biases(g, bs, s):
    """per-core masked biases: dict i -> [Cout, Rout]"""
    rows = g["rows"]
    H_of = {}
    for (name, src, wi, Cin, Cout, dil, H) in g["convs"]:
        H_of[wi] = (name, H)
    out = {}
    masks = {}
    for i, b in bs.items():
        name, H = H_of[i]
        Rn, (mult, add) = rows[name]
        lo = mult * s + add
        mask = np.array([1.0 if 0 <= lo + r < H else 0.0 for r in range(Rn)],
                        np.float32)
        out[i] = np.ascontiguousarray(
            np.asarray(b, np.float32)[:, None] * mask[None, :], np.float32)
        Cout = np.asarray(b).shape[0]
        masks[i] = np.ascontiguousarray(
            np.broadcast_to(mask[None, :], (Cout, Rn)), np.float32)
    return out, masks


def _ap_mask(g, s):
    Rn, (mult, add) = g["rows"]["ap"]
    lo = mult * s + add
    H = g["W4"]
    mask = np.array([1.0 if 0 <= lo + r < H else 0.0 for r in range(Rn)],
                    np.float32)
    return np.ascontiguousarray(
        np.broadcast_to(mask[None, :], (256, Rn)), np.float32)


def _image_slab(g, image, s):
    """image: (3, W, W). Returns zero-padded [3, Rimg, W+2] slab."""
    W = g["W"]
    Rimg, (mult, add) = g["rows"]["img"]
    lo = mult * s + add
    slab = np.zeros((3, Rimg, W + 2), np.float32)
    a = max(0, lo)
    b = min(W, lo + Rimg)
    if b > a:
        slab[:, a - lo: b - lo, 1: 1 + W] = image[:, a:b, :]
    return slab


def run_d2net(image, ws, bs, W=768, R=24, n_cores=8):
    from concourse.bass_utils import run_bass_kernel_spmd

    g = make_geometry(W, R)
    key = (W, R)
    if key not in _PROG_CACHE:
        _PROG_CACHE[key] = build_program(g)
    nc = _PROG_CACHE[key]

    starts = core_starts(g, n_cores)
    wp = _pack_weights(ws)
    in_maps = []
    for k in range(n_cores):
        s = starts[k]
        m = {"image": _image_slab(g, image, s)}
        for i in range(1, 11):
            m[f"w{i}p"] = wp[i]
        bm, mk = _masked_biases(g, bs, s)
        for i in range(1, 11):
            m[f"b{i}m"] = bm[i]
            m[f"m{i}"] = mk[i]
        m["map"] = _ap_mask(g, s)
        in_maps.append(m)

    res = run_bass_kernel_spmd(nc, in_maps, core_ids=list(range(n_cores)))

    W4 = g["W4"]
    features = np.zeros((1, 512, W4, W4), np.float32)
    detected = np.zeros((1, 512, W4, W4), np.bool_)
    displ = np.zeros((1, 2, 512, W4, W4), np.float32)
    for k in range(n_cores):
        s = starts[k]
        f = res.results[k]["features"]          # [512, R, W4]
        d = res.results[k]["detected"]          # [512, R, W4] u8
        p = res.results[k]["displ"]             # [2, 512, R, W4]
        prev_end = 0 if k == 0 else starts[k - 1] + R
        off = prev_end - s  # rows of this slab already covered by previous core
        features[0, :, s + off: s + R, :] = f[:, off:, :]
        detected[0, :, s + off: s + R, :] = d[:, off:, :].astype(np.bool_)
        displ[0, :, :, s + off: s + R, :] = p[:, :, off:, :]
    return features, detected, displ


def kernel(**inputs):
    image = np.asarray(inputs["image"], np.float32)[0]  # (3, 768, 768)
    ws = {i: np.asarray(inputs[f"w{i}"], np.float32) for i in range(1, 11)}
    bs = {i: np.asarray(inputs[f"b{i}"], np.float32) for i in range(1, 11)}
    return run_d2net(image, ws, bs, W=768, R=24, n_cores=8)
